# revision 43
# baseline (speedup 1.0000x reference)
"""BertSelfAttention (B=4, S=2048, H=768, 12 heads) on 8 TRN2 NeuronCores.

Sharding: core c -> (batch b = c//2, head-group g = c%2).  Each core computes
6 heads of one batch: Q/K/V projections restricted to that head group's 384
columns of Wq/Wk/Wv, the [S, S] score block per head, softmax, and the
context.  No cross-core communication.

Steady state is Scalar-engine limited (one exp ACTIVATE [128k, 2h, 512q] per
k-tile iteration, 192 iterations).  The schedule is a single flat stream of
192 (pair, qq, k-tile) iterations; everything else (input transposes, Q/K/V
projections) is drip-fed into the stream's PE slack so the Scalar engine
starts exp'ing at ~9 us and never starves:

  X input : 16 row-chunk DMAs [128s, 768d] f32 (contiguous), PE-transposed
            per 128x128 tile into PSUM, copy-cast f32->bf16 to SBUF X^T
            (Scalar does the first chunks' copies, DVE the rest)
  W       : f32 DMA (pair-0 e-tile of Wq/Wk first), DVE cast to bf16
  Q^T,K^T : lhsT=W tile [d,e], rhs=X^T -> PSUM [e,s]; DVE copy + bias -> bf16
  V       : lhsT=X^T tile [d,s], rhs=Wv -> PSUM [s,e]; DVE copy + bias -> bf16
            stored per head with an extra all-ones column ([V_h | 1], 65 cols)
  scores^T: lhsT=K^T_h [64,128k], rhs=Q^T_h [64,512q] -> PSUM [128k, 2, 512]
            (head pairs pack into PE rows 0-63 / 64-127 via tile_position and
            stream concurrently)
  E^T     : ScalarE exp(0.125*s + mask_k) PSUM->SBUF bf16 [128, 1024]
  ctx     : lhsT=[V_h|1] [128k, 65], rhs=E^T -> PSUM [65, q], accumulated over
            16 k-tiles; row 64 is the softmax denominator.  ctx runs at lag 2
            behind exp so the PE never waits on the current ACTIVATE.
  out     : DVE evacuate + reciprocal(denom) -> gpsimd partition_broadcast ->
            DVE mult -> SBUF f32 [64, 2048] per head -> DMA to DRAM out^T

The host transposes each core's [384, 2048] back to [2048, 384] during the
gather (pure layout).
"""

import sys

sys.path.insert(0, "/opt/trn_rl_repo")

import numpy as np

B = 4
S = 2048
HIDDEN = 768
HEADS = 12
DHEAD = 64
NCORES = 8
HLOC = 6            # heads per core
ELOC = HLOC * DHEAD  # 384 embedding columns per core
P = 128
NDT = HIDDEN // P   # 6 d-tiles (contraction)
NET = ELOC // P     # 3 e-tiles (head pairs)
NKT = S // P        # 16 k-tiles
NCH = S // P        # 16 s-chunks of X rows

_CACHE = {}

# ---- DVE fast-exp: exp(0.125*s) in 2 Vector-engine instructions ------------
# seed (stock tensor_scalar, int32 out): i = int(s*EXP_A + EXP_B); bitcast(i)
# is a Schraudolph base-2 exponential with a +-3% mantissa sawtooth.
# refine (custom op): out = y * (1 + u*(MA*u + MB)) with u = mantissa(y) - 1
# in [0,1) -- a minimax quadratic correction, max rel err 4.5e-3.  Uses only
# C0..C2 (the custom-DVE Src1 path hangs the DVE on this runtime).
EXP_A = 1512775.375            # 0.125 * log2(e) * 2^23
EXP_B = 1065352064.0           # (127 - sigma) * 2^23
EXP_MA = 0.24657432734966278
EXP_MB = -0.2419876605272293
EXP_MASK = 1.1754942106924411e-38  # f32 bits 0x007FFFFF (mantissa mask)


def _exp_ops():
    """Register the EXP_REFINE_ANT custom DVE op at runtime (idempotent)."""
    from concourse import dve_ops

    if hasattr(dve_ops, "EXP_REFINE_ANT"):
        return dve_ops.EXP_REFINE_ANT
    from concourse.dve_spec import Spec, Src0, C0, C1, C2, One, Bin, AluOp, lower
    from concourse.dve_uop import DveOpSpec

    name = "EXP_REFINE_ANT"
    _u = Bin(AluOp.BITWISE_OR, Bin(AluOp.BITWISE_AND, Src0, C0), One) - One
    body = Src0 * ((C1 * _u + C2) * _u + One)

    def _ref(in0, in1, c0, c1, c2):
        bits = np.asarray(in0).view(np.int32)
        m = ((bits & np.int32(0x007FFFFF)) | np.int32(0x3F800000)).view(np.float32)
        u = (m - np.float32(1.0)).astype(np.float32)
        return (in0 * ((np.float32(c1) * u + np.float32(c2)) * u + np.float32(1.0))).astype(
            np.float32
        )

    spec = Spec(body=body, reference=_ref)
    opcode = dve_ops._CUSTOM_DVE_ROW_BASE + len(dve_ops.OPS)
    shas = {}
    for ver in ("v3", "v4"):
        uops = lower(spec, ver=ver)
        shas[ver] = DveOpSpec(name=name, opcode=opcode, uops=uops, rd1_en=False).sha(ver)
    op = dve_ops.DveOp(name, spec, subdim=False, uops_sha=shas)
    dve_ops.OPS.append(op)
    dve_ops.CUSTOM_DVE_SPECS[name] = spec
    dve_ops._SUB_OPCODE_FOR_NAME[name] = opcode
    dve_ops.EXP_REFINE_ANT = op
    return op


def _emit(tc, aps):
    """Emit the per-core program into TileContext tc."""
    import concourse.bass as bass
    from concourse import mybir
    from concourse.masks import make_identity

    from contextlib import ExitStack

    nc = tc.nc
    f32 = mybir.dt.float32
    bf16 = mybir.dt.bfloat16
    i32 = mybir.dt.int32
    Exp = mybir.ActivationFunctionType.Exp
    ts = bass.ts
    QQ = 512                 # q-quarter width
    NIT = NET * 4 * NKT      # 192 flat iterations
    exp_op = _exp_ops()
    # iterations whose exp runs on the DVE (2-instr fast-exp) instead of the
    # Scalar engine; mid-qq positions only, none in pair-0 qq0/qq1 where the
    # DVE is busy with V/X^T work.
    # tail(qq-1) enqueues ~2.7us of DVE work at stream position 16q+3 (ctx
    # lags 4); offloads sit at +7/+11/+15 so their seed/refine never queue
    # behind that burst (the refine must beat ctx(it) at it+4).
    OFFLOAD = frozenset(
        it for it in range(32, NIT) if it % 16 in (7, 11, 15)
    )

    x, wq, wk, wv, bq, bk, bv, mask, out = (
        aps["x"], aps["wq"], aps["wk"], aps["wv"],
        aps["bq"], aps["bk"], aps["bv"], aps["mask"], aps["out"],
    )

    stack = ExitStack()
    persist = stack.enter_context(tc.tile_pool(name="persist", bufs=1))
    xr_pool = stack.enter_context(tc.tile_pool(name="xr", bufs=4))
    wstage = stack.enter_context(tc.tile_pool(name="wst", bufs=1))
    sc_pool = stack.enter_context(tc.tile_pool(name="sc", bufs=2, space="PSUM"))
    ctx_pool = stack.enter_context(tc.tile_pool(name="ctx", bufs=2, space="PSUM"))
    misc_ps = stack.enter_context(tc.tile_pool(name="mps", bufs=2, space="PSUM"))
    et_pool = stack.enter_context(tc.tile_pool(name="et", bufs=6))
    ei_pool = stack.enter_context(tc.tile_pool(name="ei", bufs=3))
    r_pool = stack.enter_context(tc.tile_pool(name="r", bufs=3))
    r0_pool = stack.enter_context(tc.tile_pool(name="r0", bufs=3))
    rbc_pool = stack.enter_context(tc.tile_pool(name="rbc", bufs=3))
    oh_pool = stack.enter_context(tc.tile_pool(name="oh", bufs=4))

    # ---- DMA plan (sync queue, in order): X chunks 0-1, pair-0 e-tile of
    # Wq/Wk, X chunks 2-3, Wv, rest of Wq/Wk, X chunks 4-15.
    xrs = []

    def x_chunk_dma(c):
        t = xr_pool.tile([P, HIDDEN], f32, tag="xr", name=f"xr{c}")
        nc.sync.dma_start(out=t[:], in_=x[ts(c, P), :])
        xrs.append(t)

    w_parts = {}

    def w_dma(name, w, e0, e1, pname):
        t = wstage.tile([P, NDT, e1 - e0], f32, tag=pname, name=pname)
        nc.sync.dma_start(
            out=t[:], in_=w[:, e0:e1].rearrange("(t p) e -> p t e", p=P)
        )
        w_parts[pname] = (t, e0, e1)

    x_chunk_dma(0)
    x_chunk_dma(1)
    w_dma("q", wq, 0, P, "wq0")
    w_dma("k", wk, 0, P, "wk0")
    x_chunk_dma(2)
    x_chunk_dma(3)
    w_dma("v", wv, 0, ELOC, "wv")
    w_dma("q", wq, P, ELOC, "wqr")
    w_dma("k", wk, P, ELOC, "wkr")
    for c in range(4, NCH):
        x_chunk_dma(c)

    # ---- W casts f32->bf16 on DVE (pair-0 tiles first) ----
    w_sb = {}
    for name in ("q", "k", "v"):
        w_sb[name] = persist.tile([P, NDT, ELOC], bf16, tag=f"w{name}", name=f"w{name}")

    def w_cast(pname, dst):
        t, e0, e1 = w_parts[pname]
        nc.vector.tensor_copy(w_sb[dst][:, :, e0:e1], t[:])

    w_cast("wq0", "q")
    w_cast("wk0", "k")

    # ---- mask/bq/bk: load as rows (contiguous, descriptor-light, SWDGE),
    # then one PE transpose into per-partition layout.
    combo = persist.tile([32, P], f32, tag="combo")
    nc.gpsimd.memset(combo[:], 0.0)
    nc.gpsimd.dma_start(out=combo[0:NKT, :], in_=mask.rearrange("(t p) -> t p", p=P))
    nc.gpsimd.dma_start(out=combo[NKT : NKT + NET, :], in_=bq.rearrange("(t p) -> t p", p=P))
    nc.gpsimd.dma_start(out=combo[NKT + NET : NKT + 2 * NET, :], in_=bk.rearrange("(t p) -> t p", p=P))
    ident32 = persist.tile([32, 32], f32, tag="id32")
    make_identity(nc, ident32[:])
    const_ps = misc_ps.tile([P, 32], f32, tag="misc", name="constps")
    nc.tensor.transpose(const_ps[:], combo[:], ident32[:])
    const_sb = persist.tile([P, 32], f32, tag="const")
    nc.vector.tensor_copy(const_sb[:], const_ps[:])
    mask_sb = const_sb[:, 0:NKT]
    bq_sb = const_sb[:, NKT : NKT + NET]
    bk_sb = const_sb[:, NKT + NET : NKT + 2 * NET]

    emask = persist.tile([P, NKT], f32, tag="emask")

    bv_row = persist.tile([1, ELOC], f32, tag="bvr")
    nc.gpsimd.dma_start(out=bv_row[:], in_=bv[None, :])
    bv_bc = persist.tile([P, ELOC], f32, tag="bvb")
    nc.gpsimd.partition_broadcast(bv_bc[:], bv_row[:])

    # ---- X^T via PE transposes: chunk c -> 6 [128,128] transposes -> PSUM,
    # copy-cast f32->bf16 to xt (Scalar for the first chunks, DVE after) ----
    ident128 = persist.tile([P, P], f32, tag="id128")
    make_identity(nc, ident128[:])
    xt = persist.tile([P, NDT, S], bf16, tag="xt")

    def t_chunk(c):
        for g in range(2):
            ps = misc_ps.tile([P, 3, P], f32, tag="misc", name=f"tr{c}_{g}")
            for j in range(3):
                nc.tensor.transpose(ps[:, j, :], xrs[c][:, ts(3 * g + j, P)], ident128[:])
            dst = xt[:, 3 * g : 3 * g + 3, ts(c, P)]
            if c < 2:
                nc.scalar.copy(dst, ps[:])
            else:
                nc.vector.tensor_copy(dst, ps[:])

    # ---- V projection s-tile: V[s, e] = X @ Wv + bv, stored [128s, 6h, 65] bf16
    v_sb = persist.tile([P, NKT, HLOC, DHEAD + 1], bf16, tag="v")

    def v_tile(st):
        vps = misc_ps.tile([P, ELOC], f32, tag="misc", name=f"vps{st}")
        for dt_i in range(NDT):
            nc.tensor.matmul(
                vps[:],
                lhsT=xt[:, dt_i, ts(st, P)],
                rhs=w_sb["v"][:, dt_i, :],
                start=(dt_i == 0),
                stop=(dt_i == NDT - 1),
            )
        nc.vector.memset(v_sb[:, st, :, DHEAD:], 1.0)  # ones column
        nc.vector.tensor_add(
            v_sb[:, st, :, 0:DHEAD],
            vps[:].rearrange("p (h d) -> p h d", d=DHEAD),
            bv_bc[:].rearrange("p (h d) -> p h d", d=DHEAD),
        )
        # fold exp(mask_k) into the V rows (incl. the ones column): exp then
        # needs no per-partition bias on either the Scalar or the DVE path.
        nc.vector.tensor_scalar_mul(
            v_sb[:, st, :, :], v_sb[:, st, :, :], emask[:, st : st + 1]
        )

    # ---- Q^T / K^T projections: [e, s] = W.T @ X^T + b ----
    qt_sb = persist.tile([P, NET, S], bf16, tag="qt")
    kt_sb = persist.tile([P, NET, S], bf16, tag="kt")

    def qk_group(proj, et_i, sb_i):
        dst, b_sb = (qt_sb, bq_sb) if proj == "q" else (kt_sb, bk_sb)
        qps = misc_ps.tile([P, QQ], f32, tag="misc", name=f"qps{proj}{et_i}_{sb_i}")
        for dt_i in range(NDT):
            nc.tensor.matmul(
                qps[:],
                lhsT=w_sb[proj][:, dt_i, ts(et_i, P)],
                rhs=xt[:, dt_i, ts(sb_i, QQ)],
                start=(dt_i == 0),
                stop=(dt_i == NDT - 1),
            )
        nc.vector.tensor_scalar_add(
            dst[:, et_i, ts(sb_i, QQ)], qps[:], b_sb[:, et_i : et_i + 1]
        )

    # ---- background work, drained into the flat iteration stream ----
    bg = []

    def add_bg(due, fn):
        bg.append((due, len(bg), fn))

    for c in range(4, NCH):              # transposes for chunks 4-15
        add_bg(c - 4, lambda cc=c: t_chunk(cc))
    for st in range(NKT):                # V tiles; ctx(st) happens at it st+2
        add_bg(st, lambda tt=st: v_tile(tt))
    for sb in (1, 2, 3):                 # K(p0) k-chunks; scores need them at
        # 4*sb, and chunk 4*sb+3's transpose lands at due 4*sb-1 (lower seq).
        add_bg(4 * sb - 1, lambda s=sb: qk_group("k", 0, s))
    for qq in (1, 2, 3):                 # Q(p0) q-chunks, >=10 iters of margin
        add_bg(max(4 * qq + 2, 16 * qq - 14), lambda q=qq: qk_group("q", 0, q))
    add_bg(16, lambda: w_cast("wqr", "q"))
    add_bg(17, lambda: w_cast("wkr", "k"))
    for pi, base in ((1, 20), (2, 72)):  # pair 1/2 projection prefetch
        jobs = [("q", 0), ("k", 0), ("k", 1), ("k", 2), ("k", 3),
                ("q", 1), ("q", 2), ("q", 3)]
        for j, (pr, sb) in enumerate(jobs):
            add_bg(base + 4 * j, lambda p=pr, e=pi, s=sb: qk_group(p, e, s))
    bg.sort(key=lambda e: (e[0], e[1]))
    bgi = [0]

    def drain(it):
        while bgi[0] < len(bg) and bg[bgi[0]][0] <= it:
            bg[bgi[0]][2]()
            bgi[0] += 1

    # ---- wv cast early (V(0) is needed at it 2); placed here so the DVE does
    # wq0/wk0/const/combo work first.
    w_cast("wv", "v")

    # ---- flat attention stream: iteration it = (pair, qq, k-tile t).
    # scores(it)+exp(it) each iteration; ctx(it-2) lags two iterations so the
    # PE never waits on the current ACTIVATE.
    ets = [None] * 6
    ctx_tiles = {}
    ohs_byp = {}

    def tail(p2, cq):
        ohs = ohs_byp[p2]
        tiles = ctx_tiles.pop((p2, cq))
        sbs = []
        for hl in range(2):  # evacuate both PSUM tiles first (frees ctx slots)
            ctx_sb = r_pool.tile([DHEAD + 1, QQ], f32, tag="r")
            nc.vector.tensor_copy(ctx_sb[:], tiles[hl][:])
            sbs.append(ctx_sb)
        for hl in range(2):
            # custom-DVE/gpsimd ops need base partition 0 on HW, so the denom
            # row is DMA-hopped to partition 0 first.
            ctx_sb = sbs[hl]
            r0 = r0_pool.tile([1, QQ], f32, tag="r0")
            nc.sync.dma_start(out=r0[:], in_=ctx_sb[DHEAD : DHEAD + 1, :])
            rr = r0_pool.tile([1, QQ], f32, tag="rr")
            nc.vector.reciprocal_approx_fast(rr[:], r0[:])
            rbc = rbc_pool.tile([DHEAD, QQ], f32, tag="rbc")
            nc.gpsimd.partition_broadcast(rbc[:], rr[:])
            nc.vector.tensor_mul(
                ohs[hl][:, ts(cq, QQ)], ctx_sb[0:DHEAD, :], rbc[:]
            )
            nc.sync.dma_start(
                out=out[ts(2 * p2 + hl, DHEAD), ts(cq, QQ)],
                in_=ohs[hl][:, ts(cq, QQ)],
            )

    def emit_ctx(ic):
        p2, rem = divmod(ic, 64)
        cq, ct = divmod(rem, 16)
        key = (p2, cq)
        if key not in ctx_tiles:
            ctx_tiles[key] = [
                ctx_pool.tile([DHEAD + 1, QQ], f32, tag="ctx", name=f"ctx{p2}_{cq}_{i}")
                for i in range(2)
            ]
        et_t = ets[ic % 6]
        for hl in range(2):
            nc.tensor.matmul(
                ctx_tiles[key][hl][:],
                lhsT=v_sb[:, ct, 2 * p2 + hl, :],
                rhs=et_t[:, hl, :],
                start=(ct == 0),
                stop=(ct == NKT - 1),
            )
        if ct == NKT - 1:
            tail(p2, cq)

    # pre-loop: first 4 chunks transposed, pair-0 first projections
    for c in range(4):
        t_chunk(c)
    qk_group("q", 0, 0)
    qk_group("k", 0, 0)
    # exp(mask): emitted after the scalar Copy instructions above so the Exp
    # activation table is loaded once and never swapped.
    nc.scalar.activation(emask[:], mask_sb, Exp, bias=0.0, scale=1.0)

    for it in range(NIT):
        p2, rem = divmod(it, 64)
        qq, t = divmod(rem, 16)
        if p2 not in ohs_byp:
            ohs_byp[p2] = [
                oh_pool.tile([DHEAD, S], f32, tag="oh", name=f"oh{p2}_{i}")
                for i in range(2)
            ]
        # ctx lags 4-5 iterations; batching two k-tiles per emission halves
        # the score<->ctx PE pipeline-transition overhead.
        if it >= 5 and it % 2 == 1:
            emit_ctx(it - 5)
            emit_ctx(it - 4)
        s_t = sc_pool.tile([P, 2, QQ], f32, tag="sc", name=f"s{it}")
        for hl in range(2):
            rows = slice(DHEAD * hl, DHEAD * (hl + 1))
            nc.tensor.matmul(
                s_t[:, hl, :],
                lhsT=kt_sb[rows, p2, ts(t, P)],
                rhs=qt_sb[rows, p2, ts(qq, QQ)],
                start=True,
                stop=True,
                tile_position=(DHEAD * hl, 0),
            )
        et_t = et_pool.tile([P, 2, QQ], bf16, tag="et", name=f"et{it}")
        ets[it % 6] = et_t
        if it in OFFLOAD:
            ei = ei_pool.tile([P, 2, QQ], i32, tag="ei", name=f"ei{it}")
            nc.vector.tensor_scalar(
                out=ei[:], in0=s_t[:], scalar1=EXP_A, scalar2=EXP_B,
                op0=mybir.AluOpType.mult, op1=mybir.AluOpType.add,
            )
            nc.vector._custom_dve(
                exp_op, out=et_t[:], in0=ei[:].bitcast(f32),
                s0=EXP_MASK, s1=EXP_MA, imm2=EXP_MB,
            )
        else:
            nc.scalar.activation(
                et_t[:], s_t[:], Exp, bias=0.0, scale=0.125,
            )
        drain(it)
    for ic in range(NIT - 4, NIT):
        emit_ctx(ic)
    drain(10 ** 9)

    stack.close()


def build():
    """Build and compile the per-core Bass program (same program on all 8 cores)."""
    if "nc" in _CACHE:
        return _CACHE["nc"]
    import concourse.bass as bass  # noqa: F401
    import concourse.tile as tile
    from concourse import bacc, mybir

    f32 = mybir.dt.float32
    nc = bacc.Bacc("TRN2", target_bir_lowering=False, debug=False, num_devices=NCORES)
    aps = {
        "x": nc.dram_tensor("x", [S, HIDDEN], f32, kind="ExternalInput").ap(),
        "wq": nc.dram_tensor("wq", [HIDDEN, ELOC], f32, kind="ExternalInput").ap(),
        "wk": nc.dram_tensor("wk", [HIDDEN, ELOC], f32, kind="ExternalInput").ap(),
        "wv": nc.dram_tensor("wv", [HIDDEN, ELOC], f32, kind="ExternalInput").ap(),
        "bq": nc.dram_tensor("bq", [ELOC], f32, kind="ExternalInput").ap(),
        "bk": nc.dram_tensor("bk", [ELOC], f32, kind="ExternalInput").ap(),
        "bv": nc.dram_tensor("bv", [ELOC], f32, kind="ExternalInput").ap(),
        "mask": nc.dram_tensor("mask", [S], f32, kind="ExternalInput").ap(),
        "out": nc.dram_tensor("out", [ELOC, S], f32, kind="ExternalOutput").ap(),
    }
    with tile.TileContext(nc) as tc:
        _emit(tc, aps)
    nc.compile()
    _CACHE["nc"] = nc
    return nc


def shard_inputs(hidden_states, attention_mask, Wq, bq, Wk, bk, Wv, bv):
    in_maps = []
    for c in range(NCORES):
        b, g = divmod(c, 2)
        cols = slice(ELOC * g, ELOC * (g + 1))
        in_maps.append({
            "x": np.ascontiguousarray(hidden_states[b], dtype=np.float32),
            "wq": np.ascontiguousarray(Wq[:, cols], dtype=np.float32),
            "wk": np.ascontiguousarray(Wk[:, cols], dtype=np.float32),
            "wv": np.ascontiguousarray(Wv[:, cols], dtype=np.float32),
            "bq": np.ascontiguousarray(bq[cols], dtype=np.float32),
            "bk": np.ascontiguousarray(bk[cols], dtype=np.float32),
            "bv": np.ascontiguousarray(bv[cols], dtype=np.float32),
            "mask": np.ascontiguousarray(
                np.asarray(attention_mask, dtype=np.float32)[b].reshape(S)
            ),
        })
    return in_maps


def gather_outputs(results):
    out = np.empty((B, S, HIDDEN), dtype=np.float32)
    for c in range(NCORES):
        b, g = divmod(c, 2)
        out[b, :, ELOC * g : ELOC * (g + 1)] = np.ascontiguousarray(results[c]["out"].T)
    return out


def kernel(**inputs):
    from concourse.bass_utils import run_bass_kernel_spmd

    nc = build()
    in_maps = shard_inputs(**{k: np.asarray(v) for k, v in inputs.items()})
    res = run_bass_kernel_spmd(nc, in_maps, list(range(NCORES)))
    return gather_outputs(res.results)


if __name__ == "__main__":
    nc = build()
    print("build + compile OK")


# revision 44
# speedup vs baseline: 1.0060x; 1.0060x over previous
"""BertSelfAttention (B=4, S=2048, H=768, 12 heads) on 8 TRN2 NeuronCores.

Sharding: core c -> (batch b = c//2, head-group g = c%2).  Each core computes
6 heads of one batch: Q/K/V projections restricted to that head group's 384
columns of Wq/Wk/Wv, the [S, S] score block per head, softmax, and the
context.  No cross-core communication.

Steady state is Scalar-engine limited (one exp ACTIVATE [128k, 2h, 512q] per
k-tile iteration, 192 iterations).  The schedule is a single flat stream of
192 (pair, qq, k-tile) iterations; everything else (input transposes, Q/K/V
projections) is drip-fed into the stream's PE slack so the Scalar engine
starts exp'ing at ~9 us and never starves:

  X input : 16 row-chunk DMAs [128s, 768d] f32 (contiguous), PE-transposed
            per 128x128 tile into PSUM, copy-cast f32->bf16 to SBUF X^T
            (Scalar does the first chunks' copies, DVE the rest)
  W       : f32 DMA (pair-0 e-tile of Wq/Wk first), DVE cast to bf16
  Q^T,K^T : lhsT=W tile [d,e], rhs=X^T -> PSUM [e,s]; DVE copy + bias -> bf16
  V       : lhsT=X^T tile [d,s], rhs=Wv -> PSUM [s,e]; DVE copy + bias -> bf16
            stored per head with an extra all-ones column ([V_h | 1], 65 cols)
  scores^T: lhsT=K^T_h [64,128k], rhs=Q^T_h [64,512q] -> PSUM [128k, 2, 512]
            (head pairs pack into PE rows 0-63 / 64-127 via tile_position and
            stream concurrently)
  E^T     : ScalarE exp(0.125*s + mask_k) PSUM->SBUF bf16 [128, 1024]
  ctx     : lhsT=[V_h|1] [128k, 65], rhs=E^T -> PSUM [65, q], accumulated over
            16 k-tiles; row 64 is the softmax denominator.  ctx runs at lag 2
            behind exp so the PE never waits on the current ACTIVATE.
  out     : DVE evacuate + reciprocal(denom) -> gpsimd partition_broadcast ->
            DVE mult -> SBUF f32 [64, 2048] per head -> DMA to DRAM out^T

The host transposes each core's [384, 2048] back to [2048, 384] during the
gather (pure layout).
"""

import sys

sys.path.insert(0, "/opt/trn_rl_repo")

import numpy as np

B = 4
S = 2048
HIDDEN = 768
HEADS = 12
DHEAD = 64
NCORES = 8
HLOC = 6            # heads per core
ELOC = HLOC * DHEAD  # 384 embedding columns per core
P = 128
NDT = HIDDEN // P   # 6 d-tiles (contraction)
NET = ELOC // P     # 3 e-tiles (head pairs)
NKT = S // P        # 16 k-tiles
NCH = S // P        # 16 s-chunks of X rows

_CACHE = {}

# ---- DVE fast-exp: exp(0.125*s) in 2 Vector-engine instructions ------------
# seed (stock tensor_scalar, int32 out): i = int(s*EXP_A + EXP_B); bitcast(i)
# is a Schraudolph base-2 exponential with a +-3% mantissa sawtooth.
# refine (custom op): out = y * (1 + u*(MA*u + MB)) with u = mantissa(y) - 1
# in [0,1) -- a minimax quadratic correction, max rel err 4.5e-3.  Uses only
# C0..C2 (the custom-DVE Src1 path hangs the DVE on this runtime).
EXP_A = 1512775.375            # 0.125 * log2(e) * 2^23
EXP_B = 1065352064.0           # (127 - sigma) * 2^23
EXP_MA = 0.24657432734966278
EXP_MB = -0.2419876605272293
EXP_MASK = 1.1754942106924411e-38  # f32 bits 0x007FFFFF (mantissa mask)


def _exp_ops():
    """Register the EXP_REFINE_ANT custom DVE op at runtime (idempotent)."""
    from concourse import dve_ops

    if hasattr(dve_ops, "EXP_REFINE_ANT"):
        return dve_ops.EXP_REFINE_ANT
    from concourse.dve_spec import Spec, Src0, C0, C1, C2, One, Bin, AluOp, lower
    from concourse.dve_uop import DveOpSpec

    name = "EXP_REFINE_ANT"
    _u = Bin(AluOp.BITWISE_OR, Bin(AluOp.BITWISE_AND, Src0, C0), One) - One
    body = Src0 * ((C1 * _u + C2) * _u + One)

    def _ref(in0, in1, c0, c1, c2):
        bits = np.asarray(in0).view(np.int32)
        m = ((bits & np.int32(0x007FFFFF)) | np.int32(0x3F800000)).view(np.float32)
        u = (m - np.float32(1.0)).astype(np.float32)
        return (in0 * ((np.float32(c1) * u + np.float32(c2)) * u + np.float32(1.0))).astype(
            np.float32
        )

    spec = Spec(body=body, reference=_ref)
    opcode = dve_ops._CUSTOM_DVE_ROW_BASE + len(dve_ops.OPS)
    shas = {}
    for ver in ("v3", "v4"):
        uops = lower(spec, ver=ver)
        shas[ver] = DveOpSpec(name=name, opcode=opcode, uops=uops, rd1_en=False).sha(ver)
    op = dve_ops.DveOp(name, spec, subdim=False, uops_sha=shas)
    dve_ops.OPS.append(op)
    dve_ops.CUSTOM_DVE_SPECS[name] = spec
    dve_ops._SUB_OPCODE_FOR_NAME[name] = opcode
    dve_ops.EXP_REFINE_ANT = op
    return op


def _emit(tc, aps):
    """Emit the per-core program into TileContext tc."""
    import concourse.bass as bass
    from concourse import mybir
    from concourse.masks import make_identity

    from contextlib import ExitStack

    nc = tc.nc
    f32 = mybir.dt.float32
    bf16 = mybir.dt.bfloat16
    i32 = mybir.dt.int32
    Exp = mybir.ActivationFunctionType.Exp
    ts = bass.ts
    QQ = 512                 # q-quarter width
    NIT = NET * 4 * NKT      # 192 flat iterations
    exp_op = _exp_ops()
    # iterations whose exp runs on the DVE (2-instr fast-exp) instead of the
    # Scalar engine; mid-qq positions only, none in pair-0 qq0/qq1 where the
    # DVE is busy with V/X^T work.
    # tail(qq-1) enqueues ~2.7us of DVE work at stream position 16q+3 (ctx
    # lags 4); offloads sit at +7/+11/+15 so their seed/refine never queue
    # behind that burst (the refine must beat ctx(it) at it+4).
    OFFLOAD = frozenset(
        it for it in range(32, NIT) if it % 16 in (7, 11, 15)
    )

    x, wq, wk, wv, bq, bk, bv, mask, out = (
        aps["x"], aps["wq"], aps["wk"], aps["wv"],
        aps["bq"], aps["bk"], aps["bv"], aps["mask"], aps["out"],
    )

    stack = ExitStack()
    persist = stack.enter_context(tc.tile_pool(name="persist", bufs=1))
    xr_pool = stack.enter_context(tc.tile_pool(name="xr", bufs=4))
    wstage = stack.enter_context(tc.tile_pool(name="wst", bufs=1))
    sc_pool = stack.enter_context(tc.tile_pool(name="sc", bufs=2, space="PSUM"))
    ctx_pool = stack.enter_context(tc.tile_pool(name="ctx", bufs=2, space="PSUM"))
    misc_ps = stack.enter_context(tc.tile_pool(name="mps", bufs=2, space="PSUM"))
    et_pool = stack.enter_context(tc.tile_pool(name="et", bufs=6))
    ei_pool = stack.enter_context(tc.tile_pool(name="ei", bufs=3))
    r_pool = stack.enter_context(tc.tile_pool(name="r", bufs=3))
    r0_pool = stack.enter_context(tc.tile_pool(name="r0", bufs=3))
    rbc_pool = stack.enter_context(tc.tile_pool(name="rbc", bufs=3))
    oh_pool = stack.enter_context(tc.tile_pool(name="oh", bufs=4))

    # ---- DMA plan (sync queue, in order): X chunks 0-1, pair-0 e-tile of
    # Wq/Wk, X chunks 2-3, Wv, rest of Wq/Wk, X chunks 4-15.
    xrs = []

    def x_chunk_dma(c):
        t = xr_pool.tile([P, HIDDEN], f32, tag="xr", name=f"xr{c}")
        nc.sync.dma_start(out=t[:], in_=x[ts(c, P), :])
        xrs.append(t)

    w_parts = {}

    def w_dma(name, w, e0, e1, pname):
        t = wstage.tile([P, NDT, e1 - e0], f32, tag=pname, name=pname)
        nc.sync.dma_start(
            out=t[:], in_=w[:, e0:e1].rearrange("(t p) e -> p t e", p=P)
        )
        w_parts[pname] = (t, e0, e1)

    x_chunk_dma(0)
    x_chunk_dma(1)
    w_dma("q", wq, 0, P, "wq0")
    w_dma("k", wk, 0, P, "wk0")
    x_chunk_dma(2)
    x_chunk_dma(3)
    w_dma("v", wv, 0, ELOC, "wv")
    w_dma("q", wq, P, ELOC, "wqr")
    w_dma("k", wk, P, ELOC, "wkr")
    for c in range(4, NCH):
        x_chunk_dma(c)

    # ---- W casts f32->bf16 on DVE (pair-0 tiles first) ----
    w_sb = {}
    for name in ("q", "k", "v"):
        w_sb[name] = persist.tile([P, NDT, ELOC], bf16, tag=f"w{name}", name=f"w{name}")

    def w_cast(pname, dst):
        t, e0, e1 = w_parts[pname]
        nc.vector.tensor_copy(w_sb[dst][:, :, e0:e1], t[:])

    w_cast("wq0", "q")
    w_cast("wk0", "k")

    # ---- mask/bq/bk: load as rows (contiguous, descriptor-light, SWDGE),
    # then one PE transpose into per-partition layout.
    combo = persist.tile([32, P], f32, tag="combo")
    nc.gpsimd.memset(combo[:], 0.0)
    nc.gpsimd.dma_start(out=combo[0:NKT, :], in_=mask.rearrange("(t p) -> t p", p=P))
    nc.gpsimd.dma_start(out=combo[NKT : NKT + NET, :], in_=bq.rearrange("(t p) -> t p", p=P))
    nc.gpsimd.dma_start(out=combo[NKT + NET : NKT + 2 * NET, :], in_=bk.rearrange("(t p) -> t p", p=P))
    ident32 = persist.tile([32, 32], f32, tag="id32")
    make_identity(nc, ident32[:])
    const_ps = misc_ps.tile([P, 32], f32, tag="misc", name="constps")
    nc.tensor.transpose(const_ps[:], combo[:], ident32[:])
    const_sb = persist.tile([P, 32], f32, tag="const")
    nc.vector.tensor_copy(const_sb[:], const_ps[:])
    mask_sb = const_sb[:, 0:NKT]
    bq_sb = const_sb[:, NKT : NKT + NET]
    bk_sb = const_sb[:, NKT + NET : NKT + 2 * NET]

    emask = persist.tile([P, NKT], f32, tag="emask")

    bv_row = persist.tile([1, ELOC], f32, tag="bvr")
    nc.gpsimd.dma_start(out=bv_row[:], in_=bv[None, :])
    bv_bc = persist.tile([P, ELOC], f32, tag="bvb")
    nc.gpsimd.partition_broadcast(bv_bc[:], bv_row[:])

    # ---- X^T via PE transposes: chunk c -> 6 [128,128] transposes -> PSUM,
    # copy-cast f32->bf16 to xt (Scalar for the first chunks, DVE after) ----
    ident128 = persist.tile([P, P], f32, tag="id128")
    make_identity(nc, ident128[:])
    xt = persist.tile([P, NDT, S], bf16, tag="xt")

    def t_chunk(c):
        for g in range(2):
            ps = misc_ps.tile([P, 3, P], f32, tag="misc", name=f"tr{c}_{g}")
            for j in range(3):
                nc.tensor.transpose(ps[:, j, :], xrs[c][:, ts(3 * g + j, P)], ident128[:])
            dst = xt[:, 3 * g : 3 * g + 3, ts(c, P)]
            if c < 2:
                nc.scalar.copy(dst, ps[:])
            else:
                nc.vector.tensor_copy(dst, ps[:])

    # ---- V projection s-tile: V[s, e] = X @ Wv + bv, stored [128s, 6h, 65] bf16
    v_sb = persist.tile([P, NKT, HLOC, DHEAD + 1], bf16, tag="v")

    def v_tile(st):
        vps = misc_ps.tile([P, ELOC], f32, tag="misc", name=f"vps{st}")
        for dt_i in range(NDT):
            nc.tensor.matmul(
                vps[:],
                lhsT=xt[:, dt_i, ts(st, P)],
                rhs=w_sb["v"][:, dt_i, :],
                start=(dt_i == 0),
                stop=(dt_i == NDT - 1),
            )
        nc.vector.memset(v_sb[:, st, :, DHEAD:], 1.0)  # ones column
        nc.vector.tensor_add(
            v_sb[:, st, :, 0:DHEAD],
            vps[:].rearrange("p (h d) -> p h d", d=DHEAD),
            bv_bc[:].rearrange("p (h d) -> p h d", d=DHEAD),
        )
        # fold exp(mask_k) into the V rows (incl. the ones column): exp then
        # needs no per-partition bias on either the Scalar or the DVE path.
        nc.vector.tensor_scalar_mul(
            v_sb[:, st, :, :], v_sb[:, st, :, :], emask[:, st : st + 1]
        )

    # ---- Q^T / K^T projections: [e, s] = W.T @ X^T + b ----
    qt_sb = persist.tile([P, NET, S], bf16, tag="qt")
    kt_sb = persist.tile([P, NET, S], bf16, tag="kt")

    def qk_group(proj, et_i, sb_i):
        dst, b_sb = (qt_sb, bq_sb) if proj == "q" else (kt_sb, bk_sb)
        qps = misc_ps.tile([P, QQ], f32, tag="misc", name=f"qps{proj}{et_i}_{sb_i}")
        for dt_i in range(NDT):
            nc.tensor.matmul(
                qps[:],
                lhsT=w_sb[proj][:, dt_i, ts(et_i, P)],
                rhs=xt[:, dt_i, ts(sb_i, QQ)],
                start=(dt_i == 0),
                stop=(dt_i == NDT - 1),
            )
        nc.vector.tensor_scalar_add(
            dst[:, et_i, ts(sb_i, QQ)], qps[:], b_sb[:, et_i : et_i + 1]
        )

    # ---- background work, drained into the flat iteration stream ----
    bg = []

    def add_bg(due, fn):
        bg.append((due, len(bg), fn))

    for c in range(4, NCH):              # transposes for chunks 4-15
        add_bg(c - 4, lambda cc=c: t_chunk(cc))
    for st in range(NKT):                # V tiles; ctx(st) happens at it st+2
        add_bg(st, lambda tt=st: v_tile(tt))
    for sb in (1, 2, 3):                 # K(p0) k-chunks; scores need them at
        # 4*sb, and chunk 4*sb+3's transpose lands at due 4*sb-1 (lower seq).
        add_bg(4 * sb - 1, lambda s=sb: qk_group("k", 0, s))
    for qq in (1, 2, 3):                 # Q(p0) q-chunks, >=10 iters of margin
        add_bg(max(4 * qq + 2, 16 * qq - 14), lambda q=qq: qk_group("q", 0, q))
    add_bg(16, lambda: w_cast("wqr", "q"))
    add_bg(17, lambda: w_cast("wkr", "k"))
    for pi, base in ((1, 20), (2, 72)):  # pair 1/2 projection prefetch
        jobs = [("q", 0), ("k", 0), ("k", 1), ("k", 2), ("k", 3),
                ("q", 1), ("q", 2), ("q", 3)]
        for j, (pr, sb) in enumerate(jobs):
            add_bg(base + 4 * j, lambda p=pr, e=pi, s=sb: qk_group(p, e, s))
    bg.sort(key=lambda e: (e[0], e[1]))
    bgi = [0]

    def drain(it):
        while bgi[0] < len(bg) and bg[bgi[0]][0] <= it:
            bg[bgi[0]][2]()
            bgi[0] += 1

    # ---- wv cast early (V(0) is needed at it 2); placed here so the DVE does
    # wq0/wk0/const/combo work first.
    w_cast("wv", "v")

    # ---- flat attention stream: iteration it = (pair, qq, k-tile t).
    # scores(it)+exp(it) each iteration; ctx(it-2) lags two iterations so the
    # PE never waits on the current ACTIVATE.
    ets = [None] * 6
    ctx_tiles = {}
    ohs_byp = {}

    def tail(p2, cq):
        ohs = ohs_byp[p2]
        tiles = ctx_tiles.pop((p2, cq))
        sbs = []
        for hl in range(2):  # evacuate both PSUM tiles first (frees ctx slots)
            ctx_sb = r_pool.tile([DHEAD + 1, QQ], f32, tag="r")
            nc.vector.tensor_copy(ctx_sb[:], tiles[hl][:])
            sbs.append(ctx_sb)
        for hl in range(2):
            # custom-DVE/gpsimd ops need base partition 0 on HW, so the denom
            # row is DMA-hopped to partition 0 first.
            ctx_sb = sbs[hl]
            r0 = r0_pool.tile([1, QQ], f32, tag="r0")
            nc.sync.dma_start(out=r0[:], in_=ctx_sb[DHEAD : DHEAD + 1, :])
            rr = r0_pool.tile([1, QQ], f32, tag="rr")
            nc.vector.reciprocal_approx_fast(rr[:], r0[:])
            rbc = rbc_pool.tile([DHEAD, QQ], f32, tag="rbc")
            nc.gpsimd.partition_broadcast(rbc[:], rr[:])
            nc.vector.tensor_mul(
                ohs[hl][:, ts(cq, QQ)], ctx_sb[0:DHEAD, :], rbc[:]
            )
            nc.sync.dma_start(
                out=out[ts(2 * p2 + hl, DHEAD), ts(cq, QQ)],
                in_=ohs[hl][:, ts(cq, QQ)],
            )

    def emit_ctx(ic):
        p2, rem = divmod(ic, 64)
        cq, ct = divmod(rem, 16)
        key = (p2, cq)
        if key not in ctx_tiles:
            ctx_tiles[key] = [
                ctx_pool.tile([DHEAD + 1, QQ], f32, tag="ctx", name=f"ctx{p2}_{cq}_{i}")
                for i in range(2)
            ]
        et_t = ets[ic % 6]
        for hl in range(2):
            nc.tensor.matmul(
                ctx_tiles[key][hl][:],
                lhsT=v_sb[:, ct, 2 * p2 + hl, :],
                rhs=et_t[:, hl, :],
                start=(ct == 0),
                stop=(ct == NKT - 1),
            )
        if ct == NKT - 1:
            tail(p2, cq)

    # pre-loop: first 4 chunks transposed, pair-0 first projections
    for c in range(4):
        t_chunk(c)
    qk_group("q", 0, 0)
    qk_group("k", 0, 0)
    # exp(mask): emitted after the scalar Copy instructions above so the Exp
    # activation table is loaded once and never swapped.
    nc.scalar.activation(emask[:], mask_sb, Exp, bias=0.0, scale=1.0)

    def emit_scores(it):
        p2, rem = divmod(it, 64)
        qq, t = divmod(rem, 16)
        s_t = sc_pool.tile([P, 2, QQ], f32, tag="sc", name=f"s{it}")
        for hl in range(2):
            rows = slice(DHEAD * hl, DHEAD * (hl + 1))
            nc.tensor.matmul(
                s_t[:, hl, :],
                lhsT=kt_sb[rows, p2, ts(t, P)],
                rhs=qt_sb[rows, p2, ts(qq, QQ)],
                start=True,
                stop=True,
                tile_position=(DHEAD * hl, 0),
            )
        return s_t

    def emit_exp(it, s_t):
        et_t = et_pool.tile([P, 2, QQ], bf16, tag="et", name=f"et{it}")
        ets[it % 6] = et_t
        if it in OFFLOAD:
            ei = ei_pool.tile([P, 2, QQ], i32, tag="ei", name=f"ei{it}")
            nc.vector.tensor_scalar(
                out=ei[:], in0=s_t[:], scalar1=EXP_A, scalar2=EXP_B,
                op0=mybir.AluOpType.mult, op1=mybir.AluOpType.add,
            )
            nc.vector._custom_dve(
                exp_op, out=et_t[:], in0=ei[:].bitcast(f32),
                s0=EXP_MASK, s1=EXP_MA, imm2=EXP_MB,
            )
        else:
            nc.scalar.activation(
                et_t[:], s_t[:], Exp, bias=0.0, scale=0.125,
            )

    # 2-iteration blocks: both score pairs back-to-back, then both exps, then
    # a 2-k-tile ctx batch (lag 4-5) — halves the PE group-transition count.
    for jb in range(NIT // 2):
        it0, it1 = 2 * jb, 2 * jb + 1
        p2 = it0 // 64
        if p2 not in ohs_byp:
            ohs_byp[p2] = [
                oh_pool.tile([DHEAD, S], f32, tag="oh", name=f"oh{p2}_{i}")
                for i in range(2)
            ]
        s0 = emit_scores(it0)
        emit_exp(it0, s0)
        s1 = emit_scores(it1)
        emit_exp(it1, s1)
        if it1 >= 5:
            emit_ctx(it1 - 5)
            emit_ctx(it1 - 4)
        drain(it0)
        drain(it1)
    for ic in range(NIT - 4, NIT):
        emit_ctx(ic)
    drain(10 ** 9)

    stack.close()


def build():
    """Build and compile the per-core Bass program (same program on all 8 cores)."""
    if "nc" in _CACHE:
        return _CACHE["nc"]
    import concourse.bass as bass  # noqa: F401
    import concourse.tile as tile
    from concourse import bacc, mybir

    f32 = mybir.dt.float32
    nc = bacc.Bacc("TRN2", target_bir_lowering=False, debug=False, num_devices=NCORES)
    aps = {
        "x": nc.dram_tensor("x", [S, HIDDEN], f32, kind="ExternalInput").ap(),
        "wq": nc.dram_tensor("wq", [HIDDEN, ELOC], f32, kind="ExternalInput").ap(),
        "wk": nc.dram_tensor("wk", [HIDDEN, ELOC], f32, kind="ExternalInput").ap(),
        "wv": nc.dram_tensor("wv", [HIDDEN, ELOC], f32, kind="ExternalInput").ap(),
        "bq": nc.dram_tensor("bq", [ELOC], f32, kind="ExternalInput").ap(),
        "bk": nc.dram_tensor("bk", [ELOC], f32, kind="ExternalInput").ap(),
        "bv": nc.dram_tensor("bv", [ELOC], f32, kind="ExternalInput").ap(),
        "mask": nc.dram_tensor("mask", [S], f32, kind="ExternalInput").ap(),
        "out": nc.dram_tensor("out", [ELOC, S], f32, kind="ExternalOutput").ap(),
    }
    with tile.TileContext(nc) as tc:
        _emit(tc, aps)
    nc.compile()
    _CACHE["nc"] = nc
    return nc


def shard_inputs(hidden_states, attention_mask, Wq, bq, Wk, bk, Wv, bv):
    in_maps = []
    for c in range(NCORES):
        b, g = divmod(c, 2)
        cols = slice(ELOC * g, ELOC * (g + 1))
        in_maps.append({
            "x": np.ascontiguousarray(hidden_states[b], dtype=np.float32),
            "wq": np.ascontiguousarray(Wq[:, cols], dtype=np.float32),
            "wk": np.ascontiguousarray(Wk[:, cols], dtype=np.float32),
            "wv": np.ascontiguousarray(Wv[:, cols], dtype=np.float32),
            "bq": np.ascontiguousarray(bq[cols], dtype=np.float32),
            "bk": np.ascontiguousarray(bk[cols], dtype=np.float32),
            "bv": np.ascontiguousarray(bv[cols], dtype=np.float32),
            "mask": np.ascontiguousarray(
                np.asarray(attention_mask, dtype=np.float32)[b].reshape(S)
            ),
        })
    return in_maps


def gather_outputs(results):
    out = np.empty((B, S, HIDDEN), dtype=np.float32)
    for c in range(NCORES):
        b, g = divmod(c, 2)
        out[b, :, ELOC * g : ELOC * (g + 1)] = np.ascontiguousarray(results[c]["out"].T)
    return out


def kernel(**inputs):
    from concourse.bass_utils import run_bass_kernel_spmd

    nc = build()
    in_maps = shard_inputs(**{k: np.asarray(v) for k, v in inputs.items()})
    res = run_bass_kernel_spmd(nc, in_maps, list(range(NCORES)))
    return gather_outputs(res.results)


if __name__ == "__main__":
    nc = build()
    print("build + compile OK")
